# revision 4
# baseline (speedup 1.0000x reference)
"""Trainium2 Bass kernel for nn_AllAtomDecoder (gnn_message_passing).

Math: all 34 side-chain atom slots of residue i are placed at CA_i, so the
[A,A] (A = L*34) radius-graph adjacency is a residue-level [L,L] adjacency
R expanded by per-atom validity vm:
    adj[(i,s),(j,t)] = R[i,j] * vm[i,s] * vm[j,t] * (1 - delta_{(i,s),(j,t)})
with R[i,i] = 1 (distance 0 < 8).  Hence
    msg[(i,s),:] = vm[i,s] * (M[i,:] - remb[i,:] - atom_sc[s,:])
where S[j,:] = cnt_j * remb[j,:] + vm[j,:] @ atom_sc   (cnt_j = sum_t vm[j,t])
      M     = R @ S                                    ([L,L] @ [L,D])

Sharding: 8 cores; cores 0-3 own batch 0, cores 4-7 batch 1; each core
computes the residue-level stages for its batch and emits 32 residues
([32, 34*128] f32) of the final output.

Layout: the per-core output [32 res, 34 t, 128 d] is computed with the
t-axis split into 4 groups (9,9,8,8) packed onto the partition axis:
partition p = 32*tg + l, free = (t_local, d).  This fills all 128
partitions so the two big DVE ops cost ~1.2k columns instead of ~4.4k.
All small inputs ride in one packed [128, 748] DMA.
"""

import numpy as np

import concourse.bass as bass
import concourse.bacc as bacc
import concourse.mybir as mybir
import concourse.tile as tile
from concourse.bass_utils import run_bass_kernel_spmd

F32 = mybir.dt.float32
ALU = mybir.AluOpType

B = 2
L = 128          # residues per batch
NCLS = 20        # enabled residue classes (>=20 are argmax-disabled)
NSC = 34         # side-chain atom slots
D = 128          # embedding dim
RPC = 32         # residues per core
NCORES = 8
R2 = 64.0        # RADIUS**2

TB = [0, 9, 18, 26]   # t-group bases
TW = [9, 9, 8, 8]     # t-group widths

# pack column offsets
_off = {}
_c = 0
for _name, _w in [("aa_f", NCLS), ("remb_f", D), ("cat_f", L), ("maskv", 1),
                  ("tbl", NSC), ("atom", D), ("aa_o4", NCLS), ("remb_o4", D),
                  ("cat_o", RPC), ("mask_o4", 1), ("eye", L)]:
    _off[_name] = _c
    _c += _w
PACKW = _c  # 748


def build_nc():
    """Build the SPMD per-core Bass graph (identical on all 8 cores)."""
    nc = bacc.Bacc("TRN2", target_bir_lowering=False, debug=False,
                   num_devices=NCORES)

    pack = nc.dram_tensor("pack", [L, PACKW], F32, kind="ExternalInput")
    atom = nc.dram_tensor("atom", [NSC, D], F32, kind="ExternalInput")
    out = nc.dram_tensor("out", [RPC, NSC * D], F32, kind="ExternalOutput")

    with tile.TileContext(nc) as tc:
        with (
            tc.tile_pool(name="sb", bufs=1) as sb,
            tc.tile_pool(name="big", bufs=1) as big,
            tc.tile_pool(name="psum", bufs=4, space=bass.MemorySpace.PSUM) as ps,
        ):
            # ---------------- loads ----------------
            pk = sb.tile([L, PACKW], F32)
            nc.sync.dma_start(pk[:], pack[:])
            aa_f_t = pk[:, _off["aa_f"]:_off["aa_f"] + NCLS]
            remb_f_t = pk[:, _off["remb_f"]:_off["remb_f"] + D]
            cat_f_t = pk[:3, _off["cat_f"]:_off["cat_f"] + L]
            maskv_t = pk[:, _off["maskv"]:_off["maskv"] + 1]
            tbl_t = pk[:NCLS, _off["tbl"]:_off["tbl"] + NSC]
            atom_t = pk[:NSC, _off["atom"]:_off["atom"] + D]
            aa_o4_t = pk[:, _off["aa_o4"]:_off["aa_o4"] + NCLS]
            remb_o4_t = pk[:, _off["remb_o4"]:_off["remb_o4"] + D]
            cat_o_t = pk[:3, _off["cat_o"]:_off["cat_o"] + RPC]
            mask_o4_t = pk[:, _off["mask_o4"]:_off["mask_o4"] + 1]
            eye_t = pk[:, _off["eye"]:_off["eye"] + L]

            # atom embedding, broadcast + t-grouped: [p=(tg,l), t_local, d]
            atom_rep = big.tile([L, 9, D], F32)
            aflat = atom[:].rearrange("t d -> (t d)")
            for tg in range(4):
                tb, tw = TB[tg], TW[tg]
                nc.sync.dma_start(
                    atom_rep[32 * tg:32 * (tg + 1), :tw, :]
                    .rearrange("l t d -> l (t d)"),
                    aflat[tb * D:(tb + tw) * D][None, :]
                    .to_broadcast((RPC, tw * D)),
                )

            ones3 = sb.tile([3, L], F32)
            nc.vector.memset(ones3[:], 1.0)

            # ---------------- one-hot (full batch + own-rows x4) ----------
            rmax_f = sb.tile([L, 1], F32)
            nc.vector.tensor_reduce(rmax_f[:], aa_f_t, op=ALU.max,
                                    axis=mybir.AxisListType.X)
            oh_f = sb.tile([L, NCLS], F32)
            nc.vector.tensor_scalar(oh_f[:], aa_f_t, rmax_f[:, :1],
                                    maskv_t, ALU.is_ge, ALU.mult)

            rmax_o = sb.tile([L, 1], F32)
            nc.vector.tensor_reduce(rmax_o[:], aa_o4_t, op=ALU.max,
                                    axis=mybir.AxisListType.X)
            oh_o4 = sb.tile([L, NCLS], F32)
            nc.vector.tensor_scalar(oh_o4[:], aa_o4_t, rmax_o[:, :1],
                                    mask_o4_t, ALU.is_ge, ALU.mult)

            ohT_f_p = ps.tile([NCLS, L], F32, tag="ps")
            nc.tensor.transpose(ohT_f_p[:], oh_f[:], eye_t)
            ohT_f = sb.tile([NCLS, L], F32)
            nc.vector.tensor_copy(ohT_f[:], ohT_f_p[:])

            ohT_o4_p = ps.tile([NCLS, L], F32, tag="ps")
            nc.tensor.transpose(ohT_o4_p[:], oh_o4[:], eye_t)
            ohT_o4 = sb.tile([NCLS, L], F32)
            nc.vector.tensor_copy(ohT_o4[:], ohT_o4_p[:])

            # ---------------- masks / S ----------------
            scmT_p = ps.tile([NSC, L], F32, tag="ps")
            nc.tensor.matmul(scmT_p[:], tbl_t, ohT_f[:])
            scmT = sb.tile([NSC, L], F32)
            nc.vector.tensor_copy(scmT[:], scmT_p[:])

            tblsum = sb.tile([NCLS, 1], F32)
            nc.vector.tensor_reduce(tblsum[:], tbl_t, op=ALU.add,
                                    axis=mybir.AxisListType.X)
            cnt_p = ps.tile([L, 1], F32, tag="ps")
            nc.tensor.matmul(cnt_p[:], ohT_f[:], tblsum[:])
            cnt = sb.tile([L, 1], F32)
            nc.vector.tensor_copy(cnt[:], cnt_p[:])

            temb_p = ps.tile([L, D], F32, tag="ps")
            nc.tensor.matmul(temb_p[:], scmT[:], atom_t)
            S_t = sb.tile([L, D], F32)
            nc.vector.scalar_tensor_tensor(S_t[:], remb_f_t, cnt[:, :1],
                                           temb_p[:], ALU.mult, ALU.add)

            # own-row side-chain mask, t-grouped: scm4[(tg,l), t_local]
            scm4_p = ps.tile([L, 9], F32, tag="ps")
            for tg in range(4):
                nc.tensor.matmul(
                    scm4_p[32 * tg:32 * (tg + 1), :TW[tg]],
                    ohT_o4[:, 32 * tg:32 * (tg + 1)],
                    tbl_t[:, TB[tg]:TB[tg] + TW[tg]],
                    tile_position=(0, 32 * tg),
                )
            scm4 = sb.tile([L, 9], F32)
            nc.vector.tensor_copy(scm4[:64, :], scm4_p[:64, :])
            nc.vector.tensor_copy(scm4[64:, :8], scm4_p[64:, :8])

            # ---------------- geometry: R columns for own rows ------------
            catsq_f = sb.tile([3, L], F32)
            nc.vector.tensor_tensor(catsq_f[:], cat_f_t, cat_f_t, op=ALU.mult)
            catm2_f = sb.tile([3, L], F32)
            nc.vector.tensor_scalar(catm2_f[:], cat_f_t, -2.0, None, ALU.mult)
            catsq_o = sb.tile([3, RPC], F32)
            nc.vector.tensor_tensor(catsq_o[:], cat_o_t, cat_o_t, op=ALU.mult)

            sqT_f_p = ps.tile([1, L], F32, tag="ps")
            nc.tensor.matmul(sqT_f_p[:], ones3[:, :1], catsq_f[:])
            sqT_f = sb.tile([1, L], F32)
            nc.vector.tensor_copy(sqT_f[:], sqT_f_p[:])

            sqT_o_p = ps.tile([1, RPC], F32, tag="ps")
            nc.tensor.matmul(sqT_o_p[:], ones3[:, :1], catsq_o[:])
            sqT_o = sb.tile([1, RPC], F32)
            nc.vector.tensor_copy(sqT_o[:], sqT_o_p[:])

            d2_p = ps.tile([L, RPC], F32, tag="ps")
            nc.tensor.matmul(d2_p[:], catm2_f[:], cat_o_t,
                             start=True, stop=False)
            nc.tensor.matmul(d2_p[:], sqT_f[:], ones3[:1, :RPC],
                             start=False, stop=False)
            nc.tensor.matmul(d2_p[:], ones3[:1, :L], sqT_o[:],
                             start=False, stop=True)

            # R columns, duplicated x4 along partitions: [(tg,l), j] layout
            # rcols4[p=(tg,l)... ] actually [j, (tg, l)] as matmul lhsT
            rcols4 = sb.tile([L, 4, RPC], F32)
            nc.vector.tensor_scalar(
                rcols4[:],
                d2_p[:, None, :].to_broadcast((L, 4, RPC)),
                R2, None, ALU.is_lt)

            # ---------------- message passing ----------------
            m4_p = ps.tile([L, D], F32, tag="ps")
            nc.tensor.matmul(m4_p[:], rcols4[:].rearrange("j a b -> j (a b)"),
                             S_t[:])
            q4 = sb.tile([L, D], F32)
            nc.vector.tensor_tensor(q4[:], m4_p[:], remb_o4_t,
                                    op=ALU.subtract)

            # ---------------- output expansion ----------------
            v4 = big.tile([L, 9, D], F32)
            nc.vector.tensor_tensor(
                v4[:, :8, :], q4[:, None, :].to_broadcast((L, 8, D)),
                atom_rep[:, :8, :], op=ALU.subtract)
            nc.vector.tensor_tensor(
                v4[:64, 8:9, :], q4[:64, None, :].to_broadcast((64, 1, D)),
                atom_rep[:64, 8:9, :], op=ALU.subtract)
            o4 = big.tile([L, 9, D], F32)
            nc.vector.tensor_tensor(
                o4[:, :8, :], v4[:, :8, :],
                scm4[:, :8, None].to_broadcast((L, 8, D)), op=ALU.mult)
            nc.vector.tensor_tensor(
                o4[:64, 8:9, :], v4[:64, 8:9, :],
                scm4[:64, 8:9, None].to_broadcast((64, 1, D)), op=ALU.mult)

            out3 = out[:].rearrange("l (t d) -> l t d", d=D)
            for tg in range(4):
                nc.sync.dma_start(
                    out3[:, TB[tg]:TB[tg] + TW[tg], :],
                    o4[32 * tg:32 * (tg + 1), :TW[tg], :])

    nc.compile()
    return nc


def make_in_maps(aa_pred, residue_embeddings, bb_pred, mask,
                 valid_atom37_mask, atom_embed):
    f32 = lambda x: np.ascontiguousarray(x, dtype=np.float32)
    eye = np.eye(L, dtype=np.float32)
    in_maps = []
    for c in range(NCORES):
        b = c // (NCORES // B)
        r0 = (c % (NCORES // B)) * RPC
        pk = np.zeros((L, PACKW), dtype=np.float32)

        def put(name, arr):
            arr = f32(arr)
            pk[:arr.shape[0], _off[name]:_off[name] + arr.shape[1]] = arr

        put("aa_f", aa_pred[b, :, :NCLS])
        put("remb_f", residue_embeddings[b])
        put("cat_f", bb_pred[b, :, 1, :].T)
        put("maskv", mask[b][:, None])
        put("tbl", valid_atom37_mask[:NCLS, 3:])
        put("atom", atom_embed[3:])
        put("aa_o4", np.tile(aa_pred[b, r0:r0 + RPC, :NCLS], (4, 1)))
        put("remb_o4", np.tile(residue_embeddings[b, r0:r0 + RPC], (4, 1)))
        put("cat_o", bb_pred[b, r0:r0 + RPC, 1, :].T)
        put("mask_o4", np.tile(mask[b, r0:r0 + RPC][:, None], (4, 1)))
        put("eye", eye)
        in_maps.append({"pack": pk, "atom": f32(atom_embed[3:])})
    return in_maps


def gather_out(results):
    chunks = [np.asarray(r["out"]).reshape(RPC, NSC, D) for r in results]
    full = np.concatenate(chunks, axis=0)          # [256, 34, 128]
    return full.reshape(B, L * NSC, D)


def kernel(**inputs) -> np.ndarray:
    nc = build_nc()
    in_maps = make_in_maps(**inputs)
    res = run_bass_kernel_spmd(nc, in_maps, core_ids=list(range(NCORES)))
    return gather_out(res.results)
